# revision 1
# baseline (speedup 1.0000x reference)
"""Trainium2 Bass kernel for BowEncoder (embedding lookup + masked mean pool).

out[b, :] = (1/len_b) * sum_{t<len_b} emb[input[b,t], :]
          = (1/len_b) * sum_v count[b, v] * emb[v, :]     (BoW form)

Sharding: vocab is split across the 8 NeuronCores (6400 zero-padded rows
each). Each core computes the partial sum over its table shard for ALL 64
batches as a dense PE matmul over 50 K-tiles of 128 vocab rows:

    psum[64, 256] += cnt_tile[128, 64].T @ emb_tile[128, 256]

Host prep per call: per-batch token histograms (uint8, exact), permuted to
the SBUF tile layout; table shard zero-padded. On device: counts arrive in
one 400KB DMA and are cast uint8->f32 once on DVE; the table shard streams
through the two HWDGE rings (SP/ACT alternating) with ramped transfer
sizes (small first groups so the first matmul starts early, 640KB groups
at steady state); all 50 matmuls accumulate into one PSUM bank; the
per-batch 1/len scale is a device-side reciprocal + per-partition
tensor_scalar; the 8 per-core partials are summed on the host (unshard).

This beats per-row gathers because SWDGE descriptor emission is serial at
~8ns/row (measured) — 16K rows/core can never beat ~130us — while the
dense stream reads the shard at ~383GB/s and the fp32 matmul runs warm at
(64+512)cyc/2.4GHz per tile.

Quirk: this walrus build allows only ONE sync-wait per instruction, so a
post-pass hoists excess waits onto same-engine NoOps.
"""

import numpy as np

import concourse.bass as bass
import concourse.mybir as mybir
import concourse.tile as tile
from concourse.bass_utils import run_bass_kernel_spmd

P = 128
B, T, V, H = 64, 2048, 50257, 256
NCORES = 8
VSHARD = 6400              # padded vocab rows per core (50 K-tiles of 128)
KT = VSHARD // P           # K-tiles per core
W = 64 + H                 # merged row width: counts | emb
KTG = 5                    # K-tiles per DMA

_DT = mybir.dt


def _split_multi_waits(nc, max_waits: int = 1) -> None:
    """This walrus build rejects instructions carrying more than one
    sync-wait. Hoist excess waits onto same-engine NoOps inserted before
    the instruction — engine queues execute in order."""
    for fn in nc.m.functions:
        for bb in fn.blocks:
            rebuilt = []
            changed = False
            for inst in bb.instructions:
                si = inst.sync_info
                if si is not None and si.on_wait and len(si.on_wait) > max_waits:
                    waits = list(si.on_wait)
                    extra, keep = waits[:-max_waits], waits[-max_waits:]
                    for j in range(0, len(extra), max_waits):
                        rebuilt.append(
                            mybir.InstNoOp(
                                name=f"{inst.name}-wsplit{j}",
                                sync_info=mybir.SyncInfo(
                                    on_wait=extra[j : j + max_waits], on_update=[]
                                ),
                                bass_nofuse=True,
                                engine=inst.engine,
                            )
                        )
                    inst.sync_info = mybir.SyncInfo(
                        on_wait=keep, on_update=list(si.on_update or [])
                    )
                    changed = True
                rebuilt.append(inst)
            if changed:
                bb.instructions = rebuilt


def _build_nc(split: bool = True):
    nc = bass.Bass("TRN2", target_bir_lowering=False)

    cnt = nc.dram_tensor("cnt", [P, KT * B], _DT.uint8, kind="ExternalInput")
    emb_t = nc.dram_tensor("embs", [VSHARD, 2 * H], _DT.bfloat16, kind="ExternalInput")
    lens = nc.dram_tensor("lens", [B, 1], _DT.int32, kind="ExternalInput")
    out = nc.dram_tensor("out", [B, H], _DT.float32, kind="ExternalOutput")

    with tile.TileContext(nc) as tc:
        with (
            tc.tile_pool(name="const", bufs=1) as const,
            tc.tile_pool(name="stream", bufs=8) as stream,
            tc.tile_pool(name="psum", bufs=1, space="PSUM") as psum_tp,
        ):
            lens_sb = const.tile([B, 1], _DT.int32)
            nc.sync.dma_start(out=lens_sb[:], in_=lens[:, :])
            lens_f = const.tile([B, 1], _DT.float32)
            nc.vector.tensor_copy(out=lens_f[:], in_=lens_sb[:])
            recip = const.tile([B, 1], _DT.float32)
            nc.vector.reciprocal(out=recip[:], in_=lens_f[:])

            # all counts up front: one 400KB DMA (host pre-permuted so
            # cnt[p, j*64+b] = count(vocab row j*128+p, batch b)), cast
            # uint8 -> f32 once on DVE
            cnt_u8 = const.tile([P, KT * B], _DT.uint8)
            nc.scalar.dma_start(out=cnt_u8[:], in_=cnt[:, :])
            cnt_f = const.tile([P, KT * B], _DT.bfloat16)
            # cast in two chunks so the first matmuls only wait on the first
            CSPLIT = 8 * B
            nc.vector.tensor_copy(out=cnt_f[:, :CSPLIT], in_=cnt_u8[:, :CSPLIT])
            nc.vector.tensor_copy(out=cnt_f[:, CSPLIT:], in_=cnt_u8[:, CSPLIT:])

            acc = psum_tp.tile([B, H], _DT.float32, space="PSUM")
            emb3 = emb_t[:, :].rearrange("(g p) h -> g p h", p=P)
            # ramped group sizes: small first transfers so the first matmul
            # starts as early as possible, big steady-state transfers after
            groups = [1, 2, 4] + [5] * 8 + [3]
            assert sum(groups) == KT
            j0 = 0
            for jg, gsz in enumerate(groups):
                tl = stream.tile([P, KTG, 2 * H], _DT.bfloat16, tag="tl")
                # alternate the two HWDGE rings (SP / ACT)
                dma_eng = nc.sync if jg % 2 == 0 else nc.scalar
                dma_eng.dma_start(
                    out=tl[:, :gsz, :],
                    in_=emb3[j0 : j0 + gsz, :, :].transpose([1, 0, 2]),
                )
                for j2 in range(gsz):
                    j = j0 + j2
                    for part in range(2):
                        nc.tensor.matmul(
                            out=acc[:],
                            lhsT=cnt_f[:, j * B : (j + 1) * B],
                            rhs=tl[:, j2, part * H : (part + 1) * H],
                            start=(j == 0 and part == 0),
                            stop=(j == KT - 1 and part == 1),
                        )
                j0 += gsz

            out_sb = const.tile([B, H], _DT.float32)
            nc.vector.tensor_scalar_mul(
                out=out_sb[:], in0=acc[:], scalar1=recip[:]
            )
            nc.sync.dma_start(out=out[:, :], in_=out_sb[:])

    if split:
        _split_multi_waits(nc)
    return nc


def _prep_in_maps(input_ids: np.ndarray, input_lens: np.ndarray, emb: np.ndarray):
    input_ids = np.asarray(input_ids, dtype=np.int64)
    input_lens = np.asarray(input_lens, dtype=np.int64)
    emb = np.asarray(emb, dtype=np.float32)

    # counts[v, b] over valid tokens
    counts = np.zeros((NCORES * VSHARD, B), dtype=np.int64)
    for b in range(B):
        L = int(input_lens[b])
        c = np.bincount(input_ids[b, :L], minlength=V)
        counts[:V, b] = c
    assert counts.max() <= 255, "uint8 count overflow"
    counts = counts.astype(np.uint8)

    import ml_dtypes

    embp = np.zeros((NCORES * VSHARD, 2 * H), dtype=ml_dtypes.bfloat16)
    hi = emb.astype(ml_dtypes.bfloat16)
    lo = (emb - hi.astype(np.float32)).astype(ml_dtypes.bfloat16)
    embp[:V, :H] = hi
    embp[:V, H:] = lo

    lens_arr = np.ascontiguousarray(input_lens.reshape(B, 1).astype(np.int32))
    in_maps = []
    for c0 in range(NCORES):
        sl = slice(c0 * VSHARD, (c0 + 1) * VSHARD)
        # cnt[p, j*64+b] = counts[shard_base + j*128 + p, b]
        cnt = np.ascontiguousarray(
            counts[sl].reshape(KT, P, B).transpose(1, 0, 2).reshape(P, KT * B)
        )
        in_maps.append(
            {"cnt": cnt, "embs": np.ascontiguousarray(embp[sl]), "lens": lens_arr}
        )
    return in_maps


_CACHE: dict = {}


def _run(inputs: dict, trace: bool = False):
    if "nc" not in _CACHE:
        _CACHE["nc"] = _build_nc()
    nc = _CACHE["nc"]
    in_maps = _prep_in_maps(inputs["input"], inputs["input_lens"], inputs["emb"])
    res = run_bass_kernel_spmd(nc, in_maps, core_ids=list(range(NCORES)), trace=trace)
    out = np.sum([res.results[c]["out"] for c in range(NCORES)], axis=0)
    return np.ascontiguousarray(out.astype(np.float32)), res


def kernel(input: np.ndarray, input_lens: np.ndarray, emb: np.ndarray) -> np.ndarray:
    out, _ = _run({"input": input, "input_lens": input_lens, "emb": emb})
    return out



# revision 2
# speedup vs baseline: 1.3120x; 1.3120x over previous
"""Trainium2 Bass kernel for BowEncoder (embedding lookup + masked mean pool).

out[b, :] = (1/len_b) * sum_{t<len_b} emb[input[b,t], :]
          = (1/len_b) * sum_v count[b, v] * emb[v, :]     (BoW form)

Sharding: vocab is split across the 8 NeuronCores (6400 zero-padded rows
each). Each core computes the partial sum over its table shard for ALL 64
batches as a dense PE matmul over 50 K-tiles of 128 vocab rows:

    psum[64, 256] += cnt_tile[128, 64].T @ emb_tile[128, 256]

v2 vs v1: the rel-err budget is 2e-2, so the table streams as a SINGLE
bf16 copy (half the HBM traffic of v1's hi+lo pair; worst-case err ~3e-3).
Counts (max 3 for this distribution) are sent as fp8e4m3 — exactly
representable — and fed straight to the PE as lhsT, eliminating the DVE
cast. 1/len is precomputed on host. The table shard is pre-transposed on
host to [128, KT*H] so each partition's DMA run is contiguous (g*512B
descriptors instead of 1KB strided rows). Ramped group sizes keep the
first matmul early; the two HWDGE rings (SP/ACT) interleave groups.

All 50 matmuls accumulate into one PSUM bank; per-batch 1/len scale is a
per-partition tensor_scalar; the 8 per-core partials are summed on the
host (unshard).

Quirk: this walrus build allows only ONE sync-wait per instruction, so a
post-pass hoists excess waits onto same-engine NoOps.
"""

import numpy as np

import concourse.bass as bass
import concourse.mybir as mybir
import concourse.tile as tile
from concourse.bass_utils import run_bass_kernel_spmd

P = 128
B, T, V, H = 64, 2048, 50257, 256
NCORES = 8
VSHARD = 6400              # padded vocab rows per core (50 K-tiles of 128)
KT = VSHARD // P           # K-tiles per core
GMAX = 10                  # max K-tiles per table DMA group

# ramped group sizes: small first transfers so the first matmul starts as
# early as possible, big steady-state transfers after
GROUPS = [1, 1, 2, 4, 6, 8, 8, 10, 10]
assert sum(GROUPS) == KT

CNT_FP8 = True             # counts as fp8e4m3 lhsT (no DVE cast)
NA = 10                    # K-tiles of counts in the first cnt DMA chunk

_DT = mybir.dt


def _split_multi_waits(nc, max_waits: int = 1) -> None:
    """This walrus build rejects instructions carrying more than one
    sync-wait. Hoist excess waits onto same-engine NoOps inserted before
    the instruction — engine queues execute in order."""
    for fn in nc.m.functions:
        for bb in fn.blocks:
            rebuilt = []
            changed = False
            for inst in bb.instructions:
                si = inst.sync_info
                if si is not None and si.on_wait and len(si.on_wait) > max_waits:
                    waits = list(si.on_wait)
                    extra, keep = waits[:-max_waits], waits[-max_waits:]
                    for j in range(0, len(extra), max_waits):
                        rebuilt.append(
                            mybir.InstNoOp(
                                name=f"{inst.name}-wsplit{j}",
                                sync_info=mybir.SyncInfo(
                                    on_wait=extra[j : j + max_waits], on_update=[]
                                ),
                                bass_nofuse=True,
                                engine=inst.engine,
                            )
                        )
                    inst.sync_info = mybir.SyncInfo(
                        on_wait=keep, on_update=list(si.on_update or [])
                    )
                    changed = True
                rebuilt.append(inst)
            if changed:
                bb.instructions = rebuilt
    return


def _build_nc(split: bool = True):
    nc = bass.Bass("TRN2", target_bir_lowering=False)

    cnt_dt = _DT.float8e4 if CNT_FP8 else _DT.uint8
    cnt = nc.dram_tensor("cnt", [P, KT * B], cnt_dt, kind="ExternalInput")
    emb_t = nc.dram_tensor("embs", [P, KT * H], _DT.bfloat16, kind="ExternalInput")
    ilen = nc.dram_tensor("ilen", [B, 1], _DT.float32, kind="ExternalInput")
    out = nc.dram_tensor("out", [B, H], _DT.float32, kind="ExternalOutput")

    with tile.TileContext(nc) as tc:
        with (
            tc.tile_pool(name="const", bufs=1) as const,
            tc.tile_pool(name="stream", bufs=5) as stream,
            tc.tile_pool(name="psum", bufs=1, space="PSUM") as psum_tp,
        ):
            # 1/len precomputed on host; tiny DMA via SWDGE so it doesn't
            # consume a HWDGE trigger slot
            ilen_sb = const.tile([B, 1], _DT.float32)
            nc.gpsimd.dma_start(out=ilen_sb[:], in_=ilen[:, :])

            # counts in two chunks so the first matmuls only wait on the
            # first NA tiles' worth
            cnt_sb = const.tile([P, KT * B], cnt_dt)
            nc.sync.dma_start(out=cnt_sb[:, : NA * B], in_=cnt[:, : NA * B])
            if CNT_FP8:
                lhs_sb = cnt_sb
            else:
                lhs_sb = const.tile([P, KT * B], _DT.bfloat16)
                nc.vector.tensor_copy(
                    out=lhs_sb[:, : NA * B], in_=cnt_sb[:, : NA * B]
                )

            acc = psum_tp.tile([B, H], _DT.float32, space="PSUM")

            j0 = 0
            cnt_tail_sent = False
            for jg, gsz in enumerate(GROUPS):
                tl = stream.tile([P, GMAX * H], _DT.bfloat16, tag="tl")
                # alternate the two HWDGE rings; g0 on ACT (sync ring is
                # busy with the cnt head chunk)
                dma_eng = nc.scalar if jg % 2 == 0 else nc.sync
                dma_eng.dma_start(
                    out=tl[:, : gsz * H],
                    in_=emb_t[:, j0 * H : (j0 + gsz) * H],
                )
                if not cnt_tail_sent:
                    # rest of the counts, second in the ACT ring's FIFO
                    nc.scalar.dma_start(
                        out=cnt_sb[:, NA * B :], in_=cnt[:, NA * B :]
                    )
                    if not CNT_FP8:
                        nc.vector.tensor_copy(
                            out=lhs_sb[:, NA * B :], in_=cnt_sb[:, NA * B :]
                        )
                    cnt_tail_sent = True
                for j2 in range(gsz):
                    j = j0 + j2
                    nc.tensor.matmul(
                        out=acc[:],
                        lhsT=lhs_sb[:, j * B : (j + 1) * B],
                        rhs=tl[:, j2 * H : (j2 + 1) * H],
                        start=(j == 0),
                        stop=(j == KT - 1),
                    )
                j0 += gsz

            out_sb = const.tile([B, H], _DT.float32)
            nc.vector.tensor_scalar_mul(
                out=out_sb[:], in0=acc[:], scalar1=ilen_sb[:]
            )
            nc.sync.dma_start(out=out[:, :], in_=out_sb[:])

    if split:
        _split_multi_waits(nc)
    return nc


def _prep_in_maps(input_ids: np.ndarray, input_lens: np.ndarray, emb: np.ndarray):
    import ml_dtypes

    input_ids = np.asarray(input_ids, dtype=np.int64)
    input_lens = np.asarray(input_lens, dtype=np.int64)
    emb = np.asarray(emb, dtype=np.float32)

    # counts[v, b] over valid tokens
    counts = np.zeros((NCORES * VSHARD, B), dtype=np.int64)
    for b in range(B):
        L = int(input_lens[b])
        c = np.bincount(input_ids[b, :L], minlength=V)
        counts[:V, b] = c
    if CNT_FP8:
        assert counts.max() <= 16, "fp8e4m3 exact-integer overflow"
        counts = counts.astype(np.float32).astype(ml_dtypes.float8_e4m3fn)
    else:
        assert counts.max() <= 255, "uint8 count overflow"
        counts = counts.astype(np.uint8)

    embp = np.zeros((NCORES * VSHARD, H), dtype=ml_dtypes.bfloat16)
    embp[:V] = emb.astype(ml_dtypes.bfloat16)

    ilen_arr = np.ascontiguousarray(
        (1.0 / input_lens.astype(np.float64)).astype(np.float32).reshape(B, 1)
    )
    in_maps = []
    for c0 in range(NCORES):
        sl = slice(c0 * VSHARD, (c0 + 1) * VSHARD)
        # cnt[p, j*64+b] = counts[shard_base + j*128 + p, b]
        cnt = np.ascontiguousarray(
            counts[sl].reshape(KT, P, B).transpose(1, 0, 2).reshape(P, KT * B)
        )
        # embs[p, j*256+h] = emb[shard_base + j*128 + p, h] — each
        # partition's stream is contiguous in DRAM
        embs = np.ascontiguousarray(
            embp[sl].reshape(KT, P, H).transpose(1, 0, 2).reshape(P, KT * H)
        )
        in_maps.append({"cnt": cnt, "embs": embs, "ilen": ilen_arr})
    return in_maps


_CACHE: dict = {}


def _run(inputs: dict, trace: bool = False):
    if "nc" not in _CACHE:
        _CACHE["nc"] = _build_nc()
    nc = _CACHE["nc"]
    in_maps = _prep_in_maps(inputs["input"], inputs["input_lens"], inputs["emb"])
    res = run_bass_kernel_spmd(nc, in_maps, core_ids=list(range(NCORES)), trace=trace)
    out = np.sum([res.results[c]["out"] for c in range(NCORES)], axis=0)
    return np.ascontiguousarray(out.astype(np.float32)), res


def kernel(input: np.ndarray, input_lens: np.ndarray, emb: np.ndarray) -> np.ndarray:
    out, _ = _run({"input": input, "input_lens": input_lens, "emb": emb})
    return out


# revision 3
# speedup vs baseline: 1.5462x; 1.1785x over previous
"""Trainium2 Bass kernel for BowEncoder (embedding lookup + masked mean pool).

out[b, :] = (1/len_b) * sum_{t<len_b} emb[input[b,t], :]
          = (1/len_b) * sum_v count[b, v] * emb[v, :]     (BoW form)

Sharding: vocab is split across the 8 NeuronCores (6400 zero-padded rows
each). Each core computes the partial sum over its table shard for ALL 64
batches as a dense PE matmul over 50 K-tiles of 128 vocab rows:

    psum[64, 256] += cnt_tile[128, 64].T @ emb_tile[128, 256]

v3: the rel-err budget is 2e-2, so the table streams as fp8 e3m4 (4
mantissa bits, 1 byte/elem — 4x less HBM traffic than v1's bf16 hi+lo).
e3m4's worst case is small-len batches (err ~ ulp/len); batches with
len <= 64 are instead computed exactly from a bf16 "repair" K-tile that
core 0 builds from their actual token rows (their counts are zeroed in
the main stream), giving global err ~2.4e-3. Counts (max 3 here) are
exact in e3m4 and ride in the SAME stream as the table: each K-tile is
320 fp8 columns = 64 counts | 256 emb, so one DMA sequence feeds both
matmul operands and there is no separate count fetch or DVE cast.
1/len is precomputed on host. The stream is pre-transposed on host so
each partition's DMA run is contiguous. Group sizes ramp up then down:
small head groups so the first matmul starts early, a small tail group
so the last matmuls retire quickly after the stream drains; the two
HWDGE rings (SP/ACT) interleave groups.

All 51 matmuls accumulate into one PSUM bank; per-batch 1/len scale is a
per-partition tensor_scalar; the 8 per-core partials are summed on the
host (unshard).

Quirk: this walrus build allows only ONE sync-wait per instruction, so a
post-pass hoists excess waits onto same-engine NoOps.
"""

import numpy as np

import concourse.bass as bass
import concourse.mybir as mybir
import concourse.tile as tile
from concourse.bass_utils import run_bass_kernel_spmd

P = 128
B, T, V, H = 64, 2048, 50257, 256
NCORES = 8
VSHARD = 6400              # padded vocab rows per core (50 K-tiles of 128)
KT = VSHARD // P           # K-tiles per core
TW = B + H                 # stream K-tile width: 64 count cols | 256 emb cols
GMAX = 10                  # max K-tiles per stream DMA group
LREP = 64                  # batches with len <= LREP go through the repair tile

# group sizes ramp up (early matmul start) then down (fast drain at the end)
GROUPS = [1, 1, 2, 4, 6, 8, 10, 10, 6, 2]
assert sum(GROUPS) == KT

_DT = mybir.dt


def _split_multi_waits(nc, max_waits: int = 1) -> None:
    """This walrus build rejects instructions carrying more than one
    sync-wait. Hoist excess waits onto same-engine NoOps inserted before
    the instruction — engine queues execute in order."""
    for fn in nc.m.functions:
        for bb in fn.blocks:
            rebuilt = []
            changed = False
            for inst in bb.instructions:
                si = inst.sync_info
                if si is not None and si.on_wait and len(si.on_wait) > max_waits:
                    waits = list(si.on_wait)
                    extra, keep = waits[:-max_waits], waits[-max_waits:]
                    for j in range(0, len(extra), max_waits):
                        rebuilt.append(
                            mybir.InstNoOp(
                                name=f"{inst.name}-wsplit{j}",
                                sync_info=mybir.SyncInfo(
                                    on_wait=extra[j : j + max_waits], on_update=[]
                                ),
                                bass_nofuse=True,
                                engine=inst.engine,
                            )
                        )
                    inst.sync_info = mybir.SyncInfo(
                        on_wait=keep, on_update=list(si.on_update or [])
                    )
                    changed = True
                rebuilt.append(inst)
            if changed:
                bb.instructions = rebuilt
    return


def _build_nc(split: bool = True):
    nc = bass.Bass("TRN2", target_bir_lowering=False)

    strm = nc.dram_tensor("strm", [P, KT * TW], _DT.float8e3, kind="ExternalInput")
    rep = nc.dram_tensor("rep", [P, TW], _DT.bfloat16, kind="ExternalInput")
    ilen = nc.dram_tensor("ilen", [B, 1], _DT.float32, kind="ExternalInput")
    out = nc.dram_tensor("out", [B, H], _DT.float32, kind="ExternalOutput")

    with tile.TileContext(nc) as tc:
        with (
            tc.tile_pool(name="const", bufs=1) as const,
            tc.tile_pool(name="stream", bufs=5) as stream,
            tc.tile_pool(name="psum", bufs=1, space="PSUM") as psum_tp,
        ):
            # 1/len precomputed on host; tiny DMA via SWDGE so it doesn't
            # consume a HWDGE trigger slot
            ilen_sb = const.tile([B, 1], _DT.float32)
            nc.gpsimd.dma_start(out=ilen_sb[:], in_=ilen[:, :])

            # bf16 repair tile (exact path for small-len batches), first
            # in the SP ring so the opening matmul unblocks early
            rep_sb = const.tile([P, TW], _DT.bfloat16)
            nc.sync.dma_start(out=rep_sb[:], in_=rep[:, :])

            acc = psum_tp.tile([B, H], _DT.float32, space="PSUM")
            nc.tensor.matmul(
                out=acc[:],
                lhsT=rep_sb[:, :B],
                rhs=rep_sb[:, B:],
                start=True,
                stop=False,
            )

            j0 = 0
            for jg, gsz in enumerate(GROUPS):
                tl = stream.tile([P, GMAX * TW], _DT.float8e3, tag="tl")
                dma_eng = nc.scalar if jg % 2 == 0 else nc.sync
                dma_eng.dma_start(
                    out=tl[:, : gsz * TW],
                    in_=strm[:, j0 * TW : (j0 + gsz) * TW],
                )
                for j2 in range(gsz):
                    j = j0 + j2
                    nc.tensor.matmul(
                        out=acc[:],
                        lhsT=tl[:, j2 * TW : j2 * TW + B],
                        rhs=tl[:, j2 * TW + B : (j2 + 1) * TW],
                        start=False,
                        stop=(j == KT - 1),
                    )
                j0 += gsz

            out_sb = const.tile([B, H], _DT.float32)
            nc.vector.tensor_scalar_mul(
                out=out_sb[:], in0=acc[:], scalar1=ilen_sb[:]
            )
            nc.sync.dma_start(out=out[:, :], in_=out_sb[:])

    if split:
        _split_multi_waits(nc)
    return nc


def _prep_in_maps(input_ids: np.ndarray, input_lens: np.ndarray, emb: np.ndarray):
    import ml_dtypes

    input_ids = np.asarray(input_ids, dtype=np.int64)
    input_lens = np.asarray(input_lens, dtype=np.int64)
    emb = np.asarray(emb, dtype=np.float32)

    # small-len batches go through the bf16 repair tile (exact), bounded
    # by its 128 rows; repair the shortest batches first
    order = np.argsort(input_lens, kind="stable")
    rep_batches = []
    budget = P
    for b in order:
        L = int(input_lens[b])
        if L > LREP or L > budget:
            break
        rep_batches.append(int(b))
        budget -= L
    rep_set = set(rep_batches)

    # counts[v, b] over valid tokens, repaired batches excluded
    counts = np.zeros((NCORES * VSHARD, B), dtype=np.int64)
    for b in range(B):
        if b in rep_set:
            continue
        L = int(input_lens[b])
        c = np.bincount(input_ids[b, :L], minlength=V)
        counts[:V, b] = c
    assert counts.max() <= 32, "e3m4 exact-integer overflow"

    # merged per-tile stream: 64 fp8 count cols | 256 fp8 emb cols
    stream = np.zeros((NCORES * VSHARD, TW), dtype=ml_dtypes.float8_e3m4)
    stream[:, :B] = counts.astype(np.float32).astype(ml_dtypes.float8_e3m4)
    stream[:V, B:] = emb.astype(ml_dtypes.float8_e3m4)

    # repair tile: one 128-row bf16 K-tile holding the repaired batches'
    # actual token rows with unit counts (core 0 only; zeros elsewhere)
    rep_tile = np.zeros((P, TW), dtype=ml_dtypes.bfloat16)
    r = 0
    for b in rep_batches:
        L = int(input_lens[b])
        rep_tile[r : r + L, b] = 1.0
        rep_tile[r : r + L, B:] = emb[input_ids[b, :L]].astype(ml_dtypes.bfloat16)
        r += L
    rep_zero = np.zeros_like(rep_tile)

    ilen_arr = np.ascontiguousarray(
        (1.0 / input_lens.astype(np.float64)).astype(np.float32).reshape(B, 1)
    )
    in_maps = []
    for c0 in range(NCORES):
        sl = slice(c0 * VSHARD, (c0 + 1) * VSHARD)
        # strm[p, j*320 + w] = stream[shard_base + j*128 + p, w] — each
        # partition's stream is contiguous in DRAM
        st = np.ascontiguousarray(
            stream[sl].reshape(KT, P, TW).transpose(1, 0, 2).reshape(P, KT * TW)
        )
        in_maps.append(
            {
                "strm": st,
                "rep": rep_tile if c0 == 0 else rep_zero,
                "ilen": ilen_arr,
            }
        )
    return in_maps


_CACHE: dict = {}


def _run(inputs: dict, trace: bool = False):
    if "nc" not in _CACHE:
        _CACHE["nc"] = _build_nc()
    nc = _CACHE["nc"]
    in_maps = _prep_in_maps(inputs["input"], inputs["input_lens"], inputs["emb"])
    res = run_bass_kernel_spmd(nc, in_maps, core_ids=list(range(NCORES)), trace=trace)
    out = np.sum([res.results[c]["out"] for c in range(NCORES)], axis=0)
    return np.ascontiguousarray(out.astype(np.float32)), res


def kernel(input: np.ndarray, input_lens: np.ndarray, emb: np.ndarray) -> np.ndarray:
    out, _ = _run({"input": input, "input_lens": input_lens, "emb": emb})
    return out
